# revision 10
# baseline (speedup 1.0000x reference)
"""ClusterAttn Trainium2 kernel (Bass/Tile), 8-way data parallel over batch.

Full inputs in, full outputs out. Internally:
  - batch B=32 is split 4-per-core across 8 NeuronCores (pure DP).
  - all GEMMs run in bf16 (fp32 psum accumulation); x is pre-cast and
    pre-transposed on the host, output is written bf16 and upcast on host.
  - act logits are computed directly from x via the host-folded
    Wfold = W_exp @ [cluster_weights*s1 | W_ga]  (contracts D=768, not
    EF=1536, and kills the fea GEMM + its 12 PE transposes per tile).
  - the cluster aggregation is re-associated: yT[d,(g,c)] = sum_s x[s,d]
    * actf[s,(g,c)] accumulated over token tiles in psum, then
    cent = sum_g yT_g^T @ W_exp_g (group-pair diagonal trick), so the
    big token contraction happens before any EF-sized GEMM.
  - attention re-associated as in the baseline: scores = x @ (Wq@k^T),
    out = attn @ (v@Wp2 + bp2)  (bp2 fold uses sum(attn)=1).
"""

from contextlib import ExitStack

import numpy as np
import ml_dtypes

import concourse.bass as bass
import concourse.bacc as bacc
import concourse.tile as tile
import concourse.mybir as mybir
from concourse import bass_utils
from concourse.masks import make_identity

dt = mybir.dt
AF = mybir.ActivationFunctionType
ALU = mybir.AluOpType

EPS = 1e-5
N_CORES = 8
B, S, D = 32, 1024, 768
E, G, C, P = 2, 8, 64, 384
EF = E * D            # 1536
GC = G * C            # 512
GFS = EF // G         # 192
NB = B // N_CORES     # batches per core
NT = S // 128         # token tiles per batch
F32 = dt.float32
BF16 = dt.bfloat16
NPBF = ml_dtypes.bfloat16


def build_program(flags):
    has_bexp, has_bq, has_bkv, has_bp2 = flags
    nc = bacc.Bacc(
        "TRN2",
        debug=False,
        enable_asserts=False,
        num_devices=N_CORES,
    )

    # x in two layouts, both bf16, host-prepared:
    #   xtok: [NB, S, D]        (token-major, for the yT GEMM lhsT side)
    #   xdt:  [NB, 128, 6, S]   (d-major: partition=d%128, free=(d//128, s))
    xtok_d = nc.dram_tensor("xtok", (NB, S, D), BF16, kind="ExternalInput").ap()
    xdt_d = nc.dram_tensor("xdt", (NB, 128, 6, S), BF16, kind="ExternalInput").ap()
    out_d = nc.dram_tensor("out", (NB, S, D), BF16, kind="ExternalOutput").ap()
    wfold_d = nc.dram_tensor("wfold", (D, GC + G), BF16, kind="ExternalInput").ap()
    wexp_d = nc.dram_tensor("wexp", (D, EF), BF16, kind="ExternalInput").ap()
    bias1_d = nc.dram_tensor("bias1", (GC + G,), F32, kind="ExternalInput").ap()
    wproj_d = nc.dram_tensor("wproj", (GFS, D), BF16, kind="ExternalInput").ap()
    s2_d = nc.dram_tensor("s2", (C, 1), F32, kind="ExternalInput").ap()
    bias2_d = nc.dram_tensor("bias2", (C, D), F32, kind="ExternalInput").ap()
    wkv_d = nc.dram_tensor("wkv", (D, 2 * P), BF16, kind="ExternalInput").ap()
    wqT_d = nc.dram_tensor("wqT", (P, D), BF16, kind="ExternalInput").ap()
    wp2_d = nc.dram_tensor("wp2", (P, D), BF16, kind="ExternalInput").ap()
    bexp_d = bq_d = bkv_d = bp2_d = None
    if has_bexp:
        bexp_d = nc.dram_tensor("bexp", (1, EF), BF16, kind="ExternalInput").ap()
    if has_bq:
        bq_d = nc.dram_tensor("bqT", (P, 1), BF16, kind="ExternalInput").ap()
    if has_bkv:
        bkv_d = nc.dram_tensor("bkv", (2 * P,), F32, kind="ExternalInput").ap()
    if has_bp2:
        bp2_d = nc.dram_tensor("bp2", (D,), F32, kind="ExternalInput").ap()

    with tile.TileContext(nc) as tc, ExitStack() as ctx:
        # ---------------- pools ----------------
        # PSUM bank budget (8 banks, one bank per tag*buf):
        #   y(2) + mm(3) + t(2) + tb2(1) = 8
        const = ctx.enter_context(tc.tile_pool(name="const", bufs=1))
        p_xt = ctx.enter_context(tc.tile_pool(name="p_xt", bufs=3))
        p_xk = ctx.enter_context(tc.tile_pool(name="p_xk", bufs=2))
        p_af = ctx.enter_context(tc.tile_pool(name="p_af", bufs=2))
        p_yt = ctx.enter_context(tc.tile_pool(name="p_yt", bufs=2))
        p_wk = ctx.enter_context(tc.tile_pool(name="p_wk", bufs=3))
        p_sm = ctx.enter_context(tc.tile_pool(name="p_sm", bufs=4))
        p_mid = ctx.enter_context(tc.tile_pool(name="p_mid", bufs=2))
        p_out = ctx.enter_context(tc.tile_pool(name="p_out", bufs=3))
        ps_y = ctx.enter_context(tc.tile_pool(name="ps_y", bufs=2, space="PSUM"))
        ps_mm = ctx.enter_context(tc.tile_pool(name="ps_mm", bufs=3, space="PSUM"))
        ps_t = ctx.enter_context(tc.tile_pool(name="ps_t", bufs=2, space="PSUM"))

        # ---------------- input prefetch ----------------
        xt_tiles = {}
        xk_tiles = {}

        def load_batch(b):
            xt = p_xt.tile([128, 6, S], BF16, tag="xt", name=f"xt{b}")
            nc.sync.dma_start(xt[:], xdt_d[b])
            xk = p_xk.tile([128, NT, D], BF16, tag="xk", name=f"xk{b}")
            nc.sync.dma_start(
                xk[:], xtok_d[b].rearrange("(t si) d -> si t d", si=128))
            xt_tiles[b] = xt
            xk_tiles[b] = xk

        # batch 0: load tile 0's xt slice first so the first act GEMM can
        # start ~3us in, instead of waiting for the full 1.5MB transfer.
        xt0 = p_xt.tile([128, 6, S], BF16, tag="xt", name="xt0")
        nc.sync.dma_start(xt0[:, :, 0:128], xdt_d[0][:, :, 0:128])
        wfold_sb = const.tile([128, 6, GC + G], BF16)
        nc.sync.dma_start(wfold_sb[:],
                          wfold_d.rearrange("(ko ki) n -> ki ko n", ki=128))
        bias1_sb = const.tile([128, GC + G], F32)
        nc.gpsimd.dma_start(bias1_sb[:], bias1_d.partition_broadcast(128))
        nc.sync.dma_start(xt0[:, :, 128:S], xdt_d[0][:, :, 128:S])
        xk0 = p_xk.tile([128, NT, D], BF16, tag="xk", name="xk0")
        nc.sync.dma_start(
            xk0[:], xtok_d[0].rearrange("(t si) d -> si t d", si=128))
        xt_tiles[0] = xt0
        xk_tiles[0] = xk0

        # ---------------- constants ----------------
        ident_f = const.tile([128, 128], F32)
        make_identity(nc, ident_f[:])
        ident = const.tile([128, 128], BF16)
        nc.vector.tensor_copy(ident[:], ident_f[:])

        load_batch(1)

        wexp_sb = const.tile([128, 6, EF], BF16)
        nc.sync.dma_start(wexp_sb[:],
                          wexp_d.rearrange("(ko ki) n -> ki ko n", ki=128))
        wproj_sb = const.tile([128, 2, D], BF16)
        nc.sync.dma_start(wproj_sb[:, 0, :], wproj_d[0:128, :])
        nc.sync.dma_start(wproj_sb[0:64, 1, :], wproj_d[128:GFS, :])
        s2_sb = const.tile([C, 1], F32)
        nc.sync.dma_start(s2_sb[:], s2_d)
        bias2_sb = const.tile([C, D], F32)
        nc.sync.dma_start(bias2_sb[:], bias2_d)
        wkv_sb = const.tile([128, 6, 2 * P], BF16)
        nc.sync.dma_start(wkv_sb[:],
                          wkv_d.rearrange("(ko ki) n -> ki ko n", ki=128))
        wqT_sb = const.tile([128, 3, D], BF16)
        nc.sync.dma_start(wqT_sb[:],
                          wqT_d.rearrange("(ko ki) n -> ki ko n", ki=128))
        wp2_sb = const.tile([128, 3, D], BF16)
        nc.sync.dma_start(wp2_sb[:],
                          wp2_d.rearrange("(ko ki) n -> ki ko n", ki=128))
        if has_bexp:
            bexp_sb = const.tile([1, EF], BF16)
            nc.sync.dma_start(bexp_sb[:], bexp_d)
            ones_tok = const.tile([128, 1], BF16)
            nc.vector.memset(ones_tok[:], 1.0)
        if has_bq:
            bq_sb = const.tile([128, 3, 1], BF16)
            nc.sync.dma_start(bq_sb[:],
                              bq_d.rearrange("(ko ki) n -> ki ko n", ki=128))
            ones_sb = const.tile([1, 128], BF16)
            nc.vector.memset(ones_sb[:], 1.0)
        if has_bkv:
            bkv_sb = const.tile([C, 2 * P], F32)
            nc.gpsimd.dma_start(bkv_sb[:], bkv_d.partition_broadcast(C))
        if has_bp2:
            bp2_sb = const.tile([C, D], F32)
            nc.gpsimd.dma_start(bp2_sb[:], bp2_d.partition_broadcast(C))

        def transpose_to(out_ps, in_ap, start=True, stop=True):
            kp = in_ap.partition_size()
            nc.tensor.matmul(
                out_ps,
                in_ap,
                ident[0:kp, 0:kp],
                is_transpose=True,
                start=start,
                stop=stop,
                skip_group_check=True,
            )

        af_tiles = {}
        yt_tiles = {}
        inv_sqrt_p = float(1.0 / np.sqrt(np.float32(P)))

        # ---------------- phase A: act logits + grouped softmax ----------
        def emit_A_tile(b, t):
            xt = xt_tiles[b]
            if t == 0:
                af_tiles[b] = p_af.tile([128, NT, GC], BF16, tag="af",
                                        name=f"af{b}")
            af = af_tiles[b]
            ts = slice(t * 128, (t + 1) * 128)

            ap_mm = ps_mm.tile([128, GC], F32, tag="mm")
            for ko in range(6):
                nc.tensor.matmul(ap_mm[:], xt[:, ko, ts],
                                 wfold_sb[:, ko, 0:GC],
                                 start=(ko == 0), stop=(ko == 5))
            gp = ps_t.tile([128, G], F32, tag="t")
            for ko in range(6):
                nc.tensor.matmul(gp[:], xt[:, ko, ts],
                                 wfold_sb[:, ko, GC:GC + G],
                                 start=(ko == 0), stop=(ko == 5))

            acts = p_wk.tile([128, GC], BF16, tag="acts")
            nc.vector.tensor_add(acts[:], ap_mm[:], bias1_sb[:, 0:GC])
            e = p_wk.tile([128, GC], BF16, tag="e")
            nc.scalar.activation(e[:], acts[:], AF.Exp)
            ssum = p_sm.tile([128, G], F32, tag="ssum")
            nc.vector.reduce_sum(ssum[:], e[:].rearrange("p (g c) -> p g c", g=G),
                                 axis=mybir.AxisListType.X)
            # gate: sigmoid via exp(-x), 1+, reciprocal (tiny tiles)
            eneg = p_sm.tile([128, G], F32, tag="eneg")
            nc.vector.tensor_add(eneg[:], gp[:], bias1_sb[:, GC:GC + G])
            nc.scalar.activation(eneg[:], eneg[:], AF.Exp, scale=-1.0)
            nc.vector.tensor_scalar_add(eneg[:], eneg[:], 1.0)
            ga = p_sm.tile([128, G], F32, tag="ga")
            nc.vector.reciprocal(ga[:], eneg[:])
            rs = p_sm.tile([128, G], F32, tag="rs")
            nc.vector.reciprocal(rs[:], ssum[:])
            nc.vector.tensor_mul(rs[:], rs[:], ga[:])
            nc.vector.tensor_tensor(
                out=af[:, t, :].rearrange("p (g c) -> p g c", g=G),
                in0=e[:].rearrange("p (g c) -> p g c", g=G),
                in1=rs[:].unsqueeze(2).broadcast_to((128, G, C)),
                op=ALU.mult)

        # ---------------- phase B: yT accumulation over token tiles ------
        def emit_B_chunk(b, ko):
            xk = xk_tiles[b]
            af = af_tiles[b]
            if ko == 0:
                yt_tiles[b] = p_yt.tile([128, 7 if has_bexp else 6, GC], BF16,
                                        tag="yt", name=f"yt{b}")
            yt = yt_tiles[b]
            ks = slice(ko * 128, (ko + 1) * 128)
            yp = ps_y.tile([128, GC], F32, tag="y")
            for t in range(NT):
                nc.tensor.matmul(yp[:], xk[:, t, ks], af[:, t, :],
                                 start=(t == 0), stop=(t == NT - 1))
            nc.scalar.copy(yt[:, ko, :], yp[:])

        def emit_B_aug(b):
            # b_exp path: column sums of actf via a ones-row contraction
            af = af_tiles[b]
            yt = yt_tiles[b]
            yp = ps_y.tile([128, GC], F32, tag="y")
            for t in range(NT):
                nc.tensor.matmul(yp[0:1, :], ones_tok[:], af[:, t, :],
                                 start=(t == 0), stop=(t == NT - 1))
            nc.scalar.copy(yt[0:1, 6, :], yp[0:1, :])

        # ---------------- mid phase: cent -> nc2 -> k,v -> wqk/vwb -------
        def emit_mid(b):
            yt = yt_tiles.pop(b)
            af_tiles.pop(b)
            xk_tiles.pop(b)

            # cent[c,f] = sum_g sum_d yT[d, g*64+c] * Wexp[d, g*192+f]
            # group-pair diagonal trick: pair j -> psum rows 0:64 = group 2j
            # (cols 0:192), rows 64:128 = group 2j+1 (cols 192:384).
            cp = ps_mm.tile([128, 384], F32, tag="mm")
            for j in range(4):
                for ko in range(6):
                    nc.tensor.matmul(
                        cp[:], yt[:, ko, j * 128:(j + 1) * 128],
                        wexp_sb[:, ko, j * 384:(j + 1) * 384],
                        start=(j == 0 and ko == 0),
                        stop=(j == 3 and ko == 5 and not has_bexp))
                if has_bexp:
                    nc.tensor.matmul(
                        cp[:], yt[0:1, 6, j * 128:(j + 1) * 128],
                        bexp_sb[0:1, j * 384:(j + 1) * 384],
                        start=False, stop=(j == 3))
            cent = p_mid.tile([C, GFS], BF16, tag="cent")
            nc.vector.tensor_copy(cent[:], cp[0:64, 0:192])
            nc.vector.tensor_add(cent[:], cent[:], cp[64:128, 192:384])

            # centT [192, 64] as [128, 2, 64]
            centT = p_mid.tile([128, 2, C], BF16, tag="centT")
            ctp = ps_t.tile([128, 384], BF16, tag="tb2", bufs=1)
            transpose_to(ctp[:, 0:64], cent[:, 0:128], start=True, stop=False)
            transpose_to(ctp[0:64, 64:128], cent[:, 128:192], start=False,
                         stop=True)
            nc.vector.tensor_copy(centT[:, 0, :], ctp[:, 0:64])
            nc.vector.tensor_copy(centT[0:64, 1, :], ctp[0:64, 64:128])

            # nc2 = BN2(cent @ W_proj + b_proj)  [64, 768]
            nc2 = p_mid.tile([C, D], BF16, tag="nc2")
            for n0, nn in ((0, 512), (512, 256)):
                np_ps = ps_mm.tile([128, 512], F32, tag="mm")
                nc.tensor.matmul(np_ps[0:C, 0:nn], centT[:, 0, :],
                                 wproj_sb[:, 0, n0:n0 + nn], start=True,
                                 stop=False)
                nc.tensor.matmul(np_ps[0:C, 0:nn], centT[0:64, 1, :],
                                 wproj_sb[0:64, 1, n0:n0 + nn], start=False,
                                 stop=True)
                nc.vector.scalar_tensor_tensor(
                    out=nc2[:, n0:n0 + nn], in0=np_ps[0:C, 0:nn],
                    scalar=s2_sb[:, 0:1], in1=bias2_sb[:, n0:n0 + nn],
                    op0=ALU.mult, op1=ALU.add)

            # nc2T [768, 64] as [128, 6, 64]
            nc2T = p_mid.tile([128, 6, C], BF16, tag="nc2T")
            ntp = ps_t.tile([128, 384], BF16, tag="tb2", bufs=1)
            for i in range(6):
                transpose_to(ntp[:, i * 64:(i + 1) * 64],
                             nc2[:, i * 128:(i + 1) * 128],
                             start=(i == 0), stop=(i == 5))
            nc.vector.tensor_copy(
                nc2T[:].rearrange("p a b -> p (a b)"), ntp[:])

            # kv = nc2 @ Wkv (+ bkv)   [64, 768]
            kv = p_mid.tile([C, 2 * P], BF16, tag="kv")
            for n0, nn in ((0, 512), (512, 256)):
                kv_ps = ps_mm.tile([128, 512], F32, tag="mm")
                for ko in range(6):
                    nc.tensor.matmul(kv_ps[0:C, 0:nn], nc2T[:, ko, :],
                                     wkv_sb[:, ko, n0:n0 + nn],
                                     start=(ko == 0), stop=(ko == 5))
                if has_bkv:
                    nc.vector.tensor_add(kv[:, n0:n0 + nn], kv_ps[0:C, 0:nn],
                                         bkv_sb[:, n0:n0 + nn])
                else:
                    nc.scalar.copy(kv[:, n0:n0 + nn], kv_ps[0:C, 0:nn])

            # kT, vT  [128, 3, 64] each (transpose k and v 128-col blocks)
            kT = p_mid.tile([128, 3, C], BF16, tag="kT")
            vT = p_mid.tile([128, 3, C], BF16, tag="vT")
            ktp = ps_t.tile([128, 384], BF16, tag="tb2", bufs=1)
            for i in range(3):
                transpose_to(ktp[:, i * 64:(i + 1) * 64],
                             kv[:, i * 128:(i + 1) * 128],
                             start=(i == 0), stop=False)
            for i in range(3):
                transpose_to(ktp[:, 192 + i * 64:192 + (i + 1) * 64],
                             kv[:, P + i * 128:P + (i + 1) * 128],
                             start=False, stop=(i == 2))
            nc.vector.tensor_copy(kT[:].rearrange("p a b -> p (a b)"),
                                  ktp[:, 0:192])
            nc.vector.tensor_copy(vT[:].rearrange("p a b -> p (a b)"),
                                  ktp[:, 192:384])

            # wqk = Wq @ k^T  [768, 64] as [128, 6, 64], all in one psum
            wqk = p_mid.tile([128, 6, C], BF16, tag="wqk")
            wq_ps = ps_t.tile([128, 384], F32, tag="tb2", bufs=1)
            for m in range(6):
                for k3 in range(3):
                    nc.tensor.matmul(wq_ps[:, m * 64:(m + 1) * 64],
                                     wqT_sb[:, k3, m * 128:(m + 1) * 128],
                                     kT[:, k3, :],
                                     start=(m == 0 and k3 == 0),
                                     stop=(m == 5 and k3 == 2))
            nc.vector.tensor_copy(wqk[:].rearrange("p a b -> p (a b)"),
                                  wq_ps[:])

            bias_c = None
            if has_bq:
                bc_ps = ps_t.tile([1, C], F32, tag="t")
                for k3 in range(3):
                    nc.tensor.matmul(bc_ps[:], bq_sb[:, k3, :], kT[:, k3, :],
                                     start=(k3 == 0), stop=(k3 == 2))
                bias_c = p_mid.tile([1, C], BF16, tag="bias_c")
                nc.scalar.copy(bias_c[:], bc_ps[:])

            # vwb = v @ Wp2 (+ bp2 folded: sum(attn) == 1)   [64, 768]
            vwb = p_mid.tile([C, D], BF16, tag="vwb")
            for n0, nn in ((0, 512), (512, 256)):
                vw_ps = ps_mm.tile([128, 512], F32, tag="mm")
                for k3 in range(3):
                    nc.tensor.matmul(vw_ps[0:C, 0:nn], vT[:, k3, :],
                                     wp2_sb[:, k3, n0:n0 + nn],
                                     start=(k3 == 0), stop=(k3 == 2))
                if has_bp2:
                    nc.vector.tensor_add(vwb[:, n0:n0 + nn], vw_ps[0:C, 0:nn],
                                         bp2_sb[:, n0:n0 + nn])
                else:
                    nc.scalar.copy(vwb[:, n0:n0 + nn], vw_ps[0:C, 0:nn])

            return {"wqk": wqk, "vwb": vwb, "bias_c": bias_c}

        # ---------------- pass 2: attention ----------------
        def emit_P2_tile(b, t, mt):
            xt = xt_tiles[b]
            wqk, vwb, bias_c = mt["wqk"], mt["vwb"], mt["bias_c"]
            ts = slice(t * 128, (t + 1) * 128)
            sc_ps = ps_t.tile([128, C], F32, tag="t")
            for ko in range(6):
                nc.tensor.matmul(sc_ps[:], xt[:, ko, ts], wqk[:, ko, :],
                                 start=(ko == 0),
                                 stop=(ko == 5 and not has_bq))
            if has_bq:
                nc.tensor.matmul(sc_ps[:], ones_sb[:], bias_c[:],
                                 start=False, stop=True)

            e_att = p_sm.tile([128, C], F32, tag="e_att")
            ssum_a = p_sm.tile([128, 1], F32, tag="ssum_a")
            nc.scalar.activation(e_att[:], sc_ps[:], AF.Exp,
                                 scale=inv_sqrt_p, accum_out=ssum_a[:])
            rs_a = p_sm.tile([128, 1], F32, tag="rs_a")
            nc.vector.reciprocal(rs_a[:], ssum_a[:])
            attn = p_sm.tile([128, C], BF16, tag="attn")
            nc.vector.tensor_scalar_mul(attn[:], e_att[:], rs_a[:])

            at_ps = ps_t.tile([128, 384], BF16, tag="tb2", bufs=1)
            transpose_to(at_ps[0:C, 0:128], attn[:])
            attnT = p_sm.tile([C, 128], BF16, tag="attnT")
            nc.vector.tensor_copy(attnT[:], at_ps[0:C, 0:128])

            outt = p_out.tile([128, D], BF16, tag="outt")
            for n0, nn in ((0, 512), (512, 256)):
                fo_ps = ps_mm.tile([128, 512], F32, tag="mm")
                nc.tensor.matmul(fo_ps[:, 0:nn], attnT[:], vwb[:, n0:n0 + nn],
                                 start=True, stop=True)
                if n0 == 0:
                    nc.vector.tensor_copy(outt[:, n0:n0 + nn], fo_ps[:, 0:nn])
                else:
                    nc.scalar.copy(outt[:, n0:n0 + nn], fo_ps[:, 0:nn])
            nc.gpsimd.dma_start(out_d[b, ts, :], outt[:])

        # ---------------- pipelined emission ----------------
        # A(b) tiles are interleaved with the previous batch's tail
        # (B chunks, mid, P2 tiles). During the last A phase we hold back
        # part of the previous tail so the final drain can interleave two
        # independent batch-chains (fills PE gaps in the serial last tail).
        def mk_tail(b):
            box = {}
            items = [lambda ko=ko: emit_B_chunk(b, ko) for ko in range(6)]
            if has_bexp:
                items.append(lambda: emit_B_aug(b))

            def do_mid():
                box["mt"] = emit_mid(b)
            items.append(do_mid)
            items += [lambda t=t: emit_P2_tile(b, t, box["mt"])
                      for t in range(NT)]
            return items

        queue = []
        for b in range(NB):
            if b + 2 < NB:
                load_batch(b + 2)
            hold = 5 if b == NB - 1 else 0
            si = 0
            per = -(-max(len(queue) - hold, 1) // NT)
            for t in range(NT):
                emit_A_tile(b, t)
                for _ in range(per):
                    if si < len(queue) - hold:
                        queue[si]()
                        si += 1
            if b < NB - 1:
                for f in queue[si:]:
                    f()
                queue = mk_tail(b)
            else:
                rest = queue[si:]
        last = mk_tail(NB - 1)
        i = j = 0
        while i < len(rest) or j < len(last):
            for _ in range(3):
                if j < len(last):
                    last[j]()
                    j += 1
            if i < len(rest):
                rest[i]()
                i += 1

    nc.compile()
    return nc


_PROGRAM_CACHE = {}


def _prep(inputs):
    """Host-side folds + bf16 casts + x pre-transpose."""
    f32 = np.float32
    g = {k: np.asarray(v, dtype=f32) for k, v in inputs.items()}
    s1 = g["bn1_g"] / np.sqrt(g["bn1_v"] + f32(EPS))
    cwf = np.concatenate([g["cluster_weights"] * s1[None, :], g["W_ga"]], axis=1)
    wfold = g["W_exp"] @ cwf                                    # (D, 520)
    bias1 = np.concatenate([g["bn1_b"] - g["bn1_m"] * s1, g["b_ga"]])
    bias1 = bias1 + g["b_exp"] @ cwf
    s2 = g["bn2_g"] / np.sqrt(g["bn2_v"] + f32(EPS))
    bias2 = (g["b_proj"][None, :] - g["bn2_m"][:, None]) * s2[:, None] + g["bn2_b"][:, None]
    flags = (
        bool(np.any(g["b_exp"])),
        bool(np.any(g["bq"])),
        bool(np.any(g["bkv"])),
        bool(np.any(g["bp2"])),
    )
    common = {
        "wfold": np.ascontiguousarray(wfold.astype(NPBF)),
        "wexp": np.ascontiguousarray(g["W_exp"].astype(NPBF)),
        "bias1": np.ascontiguousarray(bias1, dtype=f32),
        "wproj": np.ascontiguousarray(g["W_proj"].astype(NPBF)),
        "s2": np.ascontiguousarray(s2.reshape(C, 1)),
        "bias2": np.ascontiguousarray(bias2),
        "wkv": np.ascontiguousarray(g["Wkv"].astype(NPBF)),
        "wqT": np.ascontiguousarray(g["Wq"].T.astype(NPBF)),
        "wp2": np.ascontiguousarray(g["Wp2"].astype(NPBF)),
    }
    if flags[0]:
        common["bexp"] = np.ascontiguousarray(
            g["b_exp"].reshape(1, EF).astype(NPBF))
    if flags[1]:
        common["bqT"] = np.ascontiguousarray(
            g["bq"].reshape(P, 1).astype(NPBF))
    if flags[2]:
        common["bkv"] = np.ascontiguousarray(g["bkv"], dtype=f32)
    if flags[3]:
        common["bp2"] = np.ascontiguousarray(g["bp2"], dtype=f32)
    xbf = g["x"].astype(NPBF)                                   # (B, S, D)
    # d-major layout: (B, D, S) -> (B, 6, 128, S) -> (B, 128, 6, S)
    xdt = np.ascontiguousarray(
        xbf.transpose(0, 2, 1).reshape(B, 6, 128, S).transpose(0, 2, 1, 3))
    return flags, common, xbf, xdt


def run(inputs, trace=False):
    flags, common, xbf, xdt = _prep(inputs)
    if flags not in _PROGRAM_CACHE:
        _PROGRAM_CACHE[flags] = build_program(flags)
    nc = _PROGRAM_CACHE[flags]
    in_maps = []
    for c in range(N_CORES):
        m = dict(common)
        m["xtok"] = np.ascontiguousarray(xbf[c * NB:(c + 1) * NB])
        m["xdt"] = np.ascontiguousarray(xdt[c * NB:(c + 1) * NB])
        in_maps.append(m)
    res = bass_utils.run_bass_kernel_spmd(
        nc, in_maps, core_ids=list(range(N_CORES)), trace=trace)
    out = np.concatenate(
        [r["out"].astype(np.float32) for r in res.results], axis=0)
    return out, res


def kernel(**inputs):
    out, _ = run(inputs, trace=False)
    return out
